# revision 11
# baseline (speedup 1.0000x reference)
"""Trainium2 Bass kernel for attention-weighted GAP pooling (BAP).

Computes, for inputs features [B=16, C=1024, H=32, W=32] and
raw_attentions [B=16, M=32, H=32, W=32]:

    attentions     = sigmoid(raw_attentions)                      [B, M, H, W]
    feature_matrix = einsum('bchw,bmhw->bmc', features, att) / HW [B, M, C]

Sharding: data-parallel over batch, B=16 over 8 cores -> 2 samples/core.
Per core, the HW=1024 contraction runs on the tensor engine:
features are loaded naturally as [C-chunk(128 part), HW(1024 free)],
PE-transposed tile-by-tile into featT [HW-chunk(128 part), C], and
attT.T @ featT accumulates over the 8 HW-chunks into PSUM.
"""

import numpy as np

import concourse.bass as bass
import concourse.mybir as mybir
import concourse.tile as tile
from concourse import bacc
from concourse.bass_utils import run_bass_kernel_spmd
from concourse.masks import make_identity

B, C, H, W = 16, 1024, 32, 32
M = 32
HW = H * W          # 1024
NCORES = 8
BPC = B // NCORES   # 2 samples per core
P = 128
KC = HW // P        # 8 hw-chunks (contraction tiles)
CC = C // P         # 8 c-chunks
NMM = 256           # matmul n-tile width (>=256 keeps fp32r at full rate)
NT2 = C // NMM      # 4 matmul n-tiles
F32 = mybir.dt.float32
F32R = mybir.dt.float32r

# dtype knobs: "fp32" exact, "fp32r" fast PE mode for matmul and/or transpose
MM_MODE = "fp32"
TR_MODE = "fp32"


def _build(mm_mode: str = MM_MODE, tr_mode: str = TR_MODE, repeat: int = 1) -> bass.Bass:
    nc = bacc.Bacc("TRN2", target_bir_lowering=False, debug=False)
    Sig = mybir.ActivationFunctionType.Sigmoid
    Copy = mybir.ActivationFunctionType.Copy

    feat_d = nc.dram_tensor("features", [BPC, C, HW], F32, kind="ExternalInput")
    att_d = nc.dram_tensor("raw_attentions", [BPC, M, HW], F32, kind="ExternalInput")
    fm_d = nc.dram_tensor("feature_matrix", [BPC, M, C], F32, kind="ExternalOutput")
    at_d = nc.dram_tensor("attentions", [BPC, M, HW], F32, kind="ExternalOutput")

    # fp32r matmul inputs must be *produced* as float32r (the BIR verifier
    # checks producer dtype); typing the attT/featT tiles as float32r makes
    # the PSUM->SBUF copies perform the rounding for free.
    MMD = F32R if mm_mode == "fp32r" else F32

    def tr_cast(ap):
        return ap.bitcast(F32R) if tr_mode == "fp32r" else ap

    with tile.TileContext(nc) as tc:
        with (
            tc.tile_pool(name="constp", bufs=1) as constp,
            tc.tile_pool(name="attp", bufs=1) as attp,
            tc.tile_pool(name="ntp", bufs=8) as ntp,
            tc.tile_pool(name="ftp", bufs=2) as ftp,
            tc.tile_pool(name="outp", bufs=2) as outp,
            tc.tile_pool(name="pst", bufs=2, space="PSUM") as pst,
            tc.tile_pool(name="psm", bufs=3, space="PSUM") as psm,
        ):
            ident = constp.tile([P, P], F32)
            make_identity(nc, ident)

            att_sb = attp.tile([M, BPC * HW], F32)
            attT = attp.tile([P, BPC, KC, M], MMD)
            featTs = {}

            def phase_load_transpose(b):
                # load features naturally, PE-transpose to [HW-chunk, C]
                featT = ftp.tile([P, KC, C], MMD, tag="featT", name=f"featT{b}")
                featTs[b] = featT
                for c in range(CC):
                    nt = ntp.tile([P, HW], F32, tag="nt", name=f"nt{b}_{c}")
                    nc.sync.dma_start(nt, feat_d[b, c * P:(c + 1) * P, :])
                    for kq in range(KC // 4):
                        pft = pst.tile(
                            [P, 4, P], F32, tag="pft", bufs=4, name=f"pft{b}_{c}_{kq}"
                        )
                        for j in range(4):
                            k = kq * 4 + j
                            nc.tensor.transpose(
                                tr_cast(pft[:, j, :]),
                                tr_cast(nt[:, k * P:(k + 1) * P]),
                                tr_cast(ident),
                            )
                        # split each quad copy across DVE and ACT: halves the
                        # critical-path latency per quad
                        nc.vector.tensor_copy(
                            featT[:, kq * 4:kq * 4 + 2, c * P:(c + 1) * P],
                            pft[:, 0:2, :],
                        )
                        nc.scalar.copy(
                            featT[:, kq * 4 + 2:kq * 4 + 4, c * P:(c + 1) * P],
                            pft[:, 2:4, :],
                        )

            def phase_att(b):
                # sigmoid, DMA attentions out, PE-transpose att to [HW, M]
                raw = attp.tile([M, HW], F32, tag="raw", bufs=2, name=f"raw{b}")
                nc.scalar.dma_start(raw, att_d[b])
                asl = att_sb[:, b * HW:(b + 1) * HW]
                nc.scalar.activation(asl, raw, Sig)
                nc.scalar.dma_start(at_d[b], asl)
                patt = pst.tile([P, KC, M], F32, tag="patt", bufs=1, name=f"patt{b}")
                for k in range(KC):
                    nc.tensor.transpose(
                        tr_cast(patt[:, k, :]),
                        tr_cast(asl[:, k * P:(k + 1) * P]),
                        tr_cast(ident[:M, :M]),
                    )
                nc.vector.tensor_copy(attT[:, b], patt)

            def phase_matmul(b):
                featT = featTs[b]
                out_sb = outp.tile([M, C], F32, tag="out_sb", name=f"out_sb{b}")
                for n in range(NT2):
                    ps = psm.tile([M, NMM], F32, tag="mm", bufs=3, name=f"mm{b}_{n}")
                    for k in range(KC):
                        nc.tensor.matmul(
                            ps,
                            attT[:, b, k, :],
                            featT[:, k, n * NMM:(n + 1) * NMM],
                            start=(k == 0),
                            stop=(k == KC - 1),
                        )
                    nc.scalar.activation(
                        out_sb[:, n * NMM:(n + 1) * NMM], ps, Copy, scale=1.0 / HW
                    )
                nc.scalar.dma_start(fm_d[b], out_sb)

            def body(_i=None):
                phase_load_transpose(0)
                phase_att(0)
                phase_att(1)
                phase_matmul(0)
                phase_load_transpose(1)
                phase_matmul(1)

            if repeat == 1:
                body()
            else:
                # timing mode: run the whole kernel `repeat` times on-device so
                # a single execution amortizes the RPC overhead
                with tc.For_i(0, repeat, 1) as _i:
                    body(_i)

    nc.compile()
    return nc


_NC_CACHE: dict = {}


def _get_nc(mm_mode: str | None = None, tr_mode: str | None = None) -> bass.Bass:
    # read module globals at call time so tests can override MM_MODE/TR_MODE
    key = (mm_mode or MM_MODE, tr_mode or TR_MODE)
    if key not in _NC_CACHE:
        _NC_CACHE[key] = _build(*key)
    return _NC_CACHE[key]


def kernel(features: np.ndarray, raw_attentions: np.ndarray):
    """Full-input entry point: shards over 8 cores, returns full outputs."""
    features = np.ascontiguousarray(features, dtype=np.float32)
    raw_attentions = np.ascontiguousarray(raw_attentions, dtype=np.float32)
    assert features.shape == (B, C, H, W)
    assert raw_attentions.shape == (B, M, H, W)

    nc = _get_nc()
    in_maps = [
        {
            "features": features[i * BPC:(i + 1) * BPC].reshape(BPC, C, HW),
            "raw_attentions": raw_attentions[i * BPC:(i + 1) * BPC].reshape(
                BPC, M, HW
            ),
        }
        for i in range(NCORES)
    ]
    res = run_bass_kernel_spmd(nc, in_maps, core_ids=list(range(NCORES)))
    fm = np.concatenate(
        [res.results[i]["feature_matrix"] for i in range(NCORES)], axis=0
    )
    att = np.concatenate(
        [res.results[i]["attentions"] for i in range(NCORES)], axis=0
    )
    return fm.reshape(B, M, C), att.reshape(B, M, H, W)


# revision 19
# speedup vs baseline: 1.0590x; 1.0590x over previous
"""Trainium2 Bass kernel for attention-weighted GAP pooling (BAP).

Computes, for inputs features [B=16, C=1024, H=32, W=32] and
raw_attentions [B=16, M=32, H=32, W=32]:

    attentions     = sigmoid(raw_attentions)                      [B, M, H, W]
    feature_matrix = einsum('bchw,bmhw->bmc', features, att) / HW [B, M, C]

Sharding: data-parallel over batch, B=16 over 8 cores -> 2 samples/core.
Per core, the HW=1024 contraction runs on the tensor engine:
features are loaded naturally as [C-chunk(128 part), HW(1024 free)],
PE-transposed tile-by-tile into featT [HW-chunk(128 part), C], and
attT.T @ featT accumulates over the 8 HW-chunks into PSUM.
"""

import numpy as np

import concourse.bass as bass
import concourse.mybir as mybir
import concourse.tile as tile
from concourse import bacc
from concourse.bass_utils import run_bass_kernel_spmd
from concourse.masks import make_identity

B, C, H, W = 16, 1024, 32, 32
M = 32
HW = H * W          # 1024
NCORES = 8
BPC = B // NCORES   # 2 samples per core
P = 128
KC = HW // P        # 8 hw-chunks (contraction tiles)
CC = C // P         # 8 c-chunks
NMM = 256           # matmul n-tile width (>=256 keeps fp32r at full rate)
NT2 = C // NMM      # 4 matmul n-tiles
F32 = mybir.dt.float32
F32R = mybir.dt.float32r

# dtype knobs: "fp32" exact, "fp32r" fast PE mode for matmul and/or transpose.
# fp32r matmul measured on TRN2 HW: rel err 1.55e-4 (TF32-like), 4x faster
# PE streaming at N>=256. Transposes stay fp32 (exact, PE not the bottleneck).
MM_MODE = "fp32r"
TR_MODE = "fp32"


def _build(mm_mode: str = MM_MODE, tr_mode: str = TR_MODE, repeat: int = 1) -> bass.Bass:
    nc = bacc.Bacc("TRN2", target_bir_lowering=False, debug=False)
    Sig = mybir.ActivationFunctionType.Sigmoid
    Copy = mybir.ActivationFunctionType.Copy

    feat_d = nc.dram_tensor("features", [BPC, C, HW], F32, kind="ExternalInput")
    att_d = nc.dram_tensor("raw_attentions", [BPC, M, HW], F32, kind="ExternalInput")
    fm_d = nc.dram_tensor("feature_matrix", [BPC, M, C], F32, kind="ExternalOutput")
    at_d = nc.dram_tensor("attentions", [BPC, M, HW], F32, kind="ExternalOutput")

    # fp32r matmul inputs must be *produced* as float32r (the BIR verifier
    # checks producer dtype); typing the attT/featT tiles as float32r makes
    # the PSUM->SBUF copies perform the rounding for free.
    MMD = F32R if mm_mode == "fp32r" else F32

    def tr_cast(ap):
        return ap.bitcast(F32R) if tr_mode == "fp32r" else ap

    with tile.TileContext(nc) as tc:
        with (
            tc.tile_pool(name="constp", bufs=1) as constp,
            tc.tile_pool(name="attp", bufs=1) as attp,
            tc.tile_pool(name="ntp", bufs=8) as ntp,
            tc.tile_pool(name="ftp", bufs=2) as ftp,
            tc.tile_pool(name="outp", bufs=2) as outp,
            tc.tile_pool(name="pst", bufs=2, space="PSUM") as pst,
            tc.tile_pool(name="psm", bufs=3, space="PSUM") as psm,
        ):
            ident = constp.tile([P, P], F32)
            make_identity(nc, ident)

            # PE warm-up: dummy transposes during the DMA head so the PE is
            # at full clock (HAM ramped) when the first real tiles arrive
            wps = pst.tile([P, 4, P], F32, tag="pft", bufs=4, name="wps")
            for _w in range(14):
                nc.tensor.transpose(wps[:, _w % 4, :], ident, ident)

            att_sb = attp.tile([M, BPC * HW], F32)
            attT = attp.tile([P, BPC, KC, M], MMD)
            featTs = {}

            def phase_load_transpose(b):
                # load features naturally, PE-transpose to [HW-chunk, C]
                featT = ftp.tile([P, KC, C], MMD, tag="featT", name=f"featT{b}")
                featTs[b] = featT
                for c in range(CC):
                    nt = ntp.tile([P, HW], F32, tag="nt", name=f"nt{b}_{c}")
                    hh = HW // 2
                    nc.sync.dma_start(nt[:, :hh], feat_d[b, c * P:(c + 1) * P, :hh])
                    nc.sync.dma_start(nt[:, hh:], feat_d[b, c * P:(c + 1) * P, hh:])
                    for kq in range(KC // 4):
                        pft = pst.tile(
                            [P, 4, P], F32, tag="pft", bufs=4, name=f"pft{b}_{c}_{kq}"
                        )
                        for j in range(4):
                            k = kq * 4 + j
                            nc.tensor.transpose(
                                tr_cast(pft[:, j, :]),
                                tr_cast(nt[:, k * P:(k + 1) * P]),
                                tr_cast(ident),
                            )
                        # split each quad copy across DVE and ACT: halves the
                        # critical-path latency per quad
                        nc.vector.tensor_copy(
                            featT[:, kq * 4:kq * 4 + 2, c * P:(c + 1) * P],
                            pft[:, 0:2, :],
                        )
                        nc.scalar.copy(
                            featT[:, kq * 4 + 2:kq * 4 + 4, c * P:(c + 1) * P],
                            pft[:, 2:4, :],
                        )

            def phase_att(b):
                # sigmoid, DMA attentions out, PE-transpose att to [HW, M]
                raw = attp.tile([M, HW], F32, tag="raw", bufs=2, name=f"raw{b}")
                nc.scalar.dma_start(raw, att_d[b])
                asl = att_sb[:, b * HW:(b + 1) * HW]
                nc.scalar.activation(asl, raw, Sig)
                nc.scalar.dma_start(at_d[b], asl)
                patt = pst.tile([P, KC, M], F32, tag="patt", bufs=1, name=f"patt{b}")
                for k in range(KC):
                    nc.tensor.transpose(
                        tr_cast(patt[:, k, :]),
                        tr_cast(asl[:, k * P:(k + 1) * P]),
                        tr_cast(ident[:M, :M]),
                    )
                nc.vector.tensor_copy(attT[:, b], patt)

            def phase_matmul(b):
                # last sample's output DMA goes via the SP queue, which is idle
                # by then (input loads done) -> minimal enqueue latency
                out_eng = nc.sync if b == BPC - 1 else nc.scalar
                featT = featTs[b]
                out_sb = outp.tile([M, C], F32, tag="out_sb", name=f"out_sb{b}")
                for n in range(NT2):
                    ps = psm.tile([M, NMM], F32, tag="mm", bufs=3, name=f"mm{b}_{n}")
                    for k in range(KC):
                        nc.tensor.matmul(
                            ps,
                            attT[:, b, k, :],
                            featT[:, k, n * NMM:(n + 1) * NMM],
                            start=(k == 0),
                            stop=(k == KC - 1),
                        )
                    nc.scalar.activation(
                        out_sb[:, n * NMM:(n + 1) * NMM], ps, Copy, scale=1.0 / HW
                    )
                    if n % 2 == 1:
                        out_eng.dma_start(
                            fm_d[b, :, (n - 1) * NMM:(n + 1) * NMM],
                            out_sb[:, (n - 1) * NMM:(n + 1) * NMM],
                        )

            def body(_i=None):
                phase_load_transpose(0)
                phase_att(0)
                phase_att(1)
                phase_matmul(0)
                phase_load_transpose(1)
                phase_matmul(1)

            if repeat == 1:
                body()
            else:
                # timing mode: run the whole kernel `repeat` times on-device so
                # a single execution amortizes the RPC overhead
                with tc.For_i(0, repeat, 1) as _i:
                    body(_i)

    nc.compile()
    return nc


_NC_CACHE: dict = {}


def _get_nc(mm_mode: str | None = None, tr_mode: str | None = None) -> bass.Bass:
    # read module globals at call time so tests can override MM_MODE/TR_MODE
    key = (mm_mode or MM_MODE, tr_mode or TR_MODE)
    if key not in _NC_CACHE:
        _NC_CACHE[key] = _build(*key)
    return _NC_CACHE[key]


def kernel(features: np.ndarray, raw_attentions: np.ndarray):
    """Full-input entry point: shards over 8 cores, returns full outputs."""
    features = np.ascontiguousarray(features, dtype=np.float32)
    raw_attentions = np.ascontiguousarray(raw_attentions, dtype=np.float32)
    assert features.shape == (B, C, H, W)
    assert raw_attentions.shape == (B, M, H, W)

    nc = _get_nc()
    in_maps = [
        {
            "features": features[i * BPC:(i + 1) * BPC].reshape(BPC, C, HW),
            "raw_attentions": raw_attentions[i * BPC:(i + 1) * BPC].reshape(
                BPC, M, HW
            ),
        }
        for i in range(NCORES)
    ]
    res = run_bass_kernel_spmd(nc, in_maps, core_ids=list(range(NCORES)))
    fm = np.concatenate(
        [res.results[i]["feature_matrix"] for i in range(NCORES)], axis=0
    )
    att = np.concatenate(
        [res.results[i]["attentions"] for i in range(NCORES)], axis=0
    )
    return fm.reshape(B, M, C), att.reshape(B, M, H, W)


# revision 22
# speedup vs baseline: 1.0702x; 1.0106x over previous
"""Trainium2 Bass kernel for attention-weighted GAP pooling (BAP).

Computes, for inputs features [B=16, C=1024, H=32, W=32] and
raw_attentions [B=16, M=32, H=32, W=32]:

    attentions     = sigmoid(raw_attentions)                      [B, M, H, W]
    feature_matrix = einsum('bchw,bmhw->bmc', features, att) / HW [B, M, C]

Sharding: data-parallel over batch, B=16 over 8 cores -> 2 samples/core.
Per core, the HW=1024 contraction runs on the tensor engine:
features are loaded naturally as [C-chunk(128 part), HW(1024 free)],
PE-transposed tile-by-tile into featT [HW-chunk(128 part), C], and
attT.T @ featT accumulates over the 8 HW-chunks into PSUM (fp32 accumulate).
Matmul inputs use float32r (full-rate PE streaming at N>=256; the PSUM->SBUF
copies perform the required fp32r rounding); transposes stay exact fp32.

Measured on HW (8 cores, in-NEFF For_i loop, delta method): ~37.8 us/iter
incl ~2us loop back-edge; cost-model estimate 34.0 us. DMA roofline (8.9 MB
per core at ~350 GB/s) is ~26 us. Output rel err vs fp32 reference: 1.6e-4
(fp32r matmul rounding); set MM_MODE="fp32" for exact fp32 (~54 us).
"""

import numpy as np

import concourse.bass as bass
import concourse.mybir as mybir
import concourse.tile as tile
from concourse import bacc
from concourse.bass_utils import run_bass_kernel_spmd
from concourse.masks import make_identity

B, C, H, W = 16, 1024, 32, 32
M = 32
HW = H * W          # 1024
NCORES = 8
BPC = B // NCORES   # 2 samples per core
P = 128
KC = HW // P        # 8 hw-chunks (contraction tiles)
CC = C // P         # 8 c-chunks
NMM = 256           # matmul n-tile width (>=256 keeps fp32r at full rate)
NT2 = C // NMM      # 4 matmul n-tiles
F32 = mybir.dt.float32
F32R = mybir.dt.float32r

# dtype knobs: "fp32" exact, "fp32r" fast PE mode for matmul and/or transpose.
# fp32r matmul measured on TRN2 HW: rel err 1.55e-4 (TF32-like), 4x faster
# PE streaming at N>=256. Transposes stay fp32 (exact, PE not the bottleneck).
MM_MODE = "fp32r"
TR_MODE = "fp32"


def _build(mm_mode: str = MM_MODE, tr_mode: str = TR_MODE, repeat: int = 1) -> bass.Bass:
    nc = bacc.Bacc("TRN2", target_bir_lowering=False, debug=False)
    Sig = mybir.ActivationFunctionType.Sigmoid
    Copy = mybir.ActivationFunctionType.Copy

    feat_d = nc.dram_tensor("features", [BPC, C, HW], F32, kind="ExternalInput")
    att_d = nc.dram_tensor("raw_attentions", [BPC, M, HW], F32, kind="ExternalInput")
    fm_d = nc.dram_tensor("feature_matrix", [BPC, M, C], F32, kind="ExternalOutput")
    at_d = nc.dram_tensor("attentions", [BPC, M, HW], F32, kind="ExternalOutput")

    # fp32r matmul inputs must be *produced* as float32r (the BIR verifier
    # checks producer dtype); typing the attT/featT tiles as float32r makes
    # the PSUM->SBUF copies perform the rounding for free.
    MMD = F32R if mm_mode == "fp32r" else F32

    def tr_cast(ap):
        return ap.bitcast(F32R) if tr_mode == "fp32r" else ap

    with tile.TileContext(nc) as tc:
        with (
            tc.tile_pool(name="constp", bufs=1) as constp,
            tc.tile_pool(name="attp", bufs=1) as attp,
            tc.tile_pool(name="ntp", bufs=8) as ntp,
            tc.tile_pool(name="ftp", bufs=2) as ftp,
            tc.tile_pool(name="outp", bufs=2) as outp,
            tc.tile_pool(name="pst", bufs=2, space="PSUM") as pst,
            tc.tile_pool(name="psm", bufs=3, space="PSUM") as psm,
        ):
            ident = constp.tile([P, P], F32)
            make_identity(nc, ident)

            # PE warm-up: dummy transposes during the DMA head so the PE is
            # at full clock (HAM ramped) when the first real tiles arrive
            wps = pst.tile([P, 4, P], F32, tag="pft", bufs=5, name="wps")
            for _w in range(14):
                nc.tensor.transpose(wps[:, _w % 4, :], ident, ident)

            att_sb = attp.tile([M, BPC * HW], F32)
            attT = attp.tile([P, BPC, KC, M], MMD)
            featTs = {}

            def phase_load_transpose(b):
                # load features naturally, PE-transpose to [HW-chunk, C]
                featT = ftp.tile([P, KC, C], MMD, tag="featT", name=f"featT{b}")
                featTs[b] = featT
                for c in range(CC):
                    nt = ntp.tile([P, HW], F32, tag="nt", name=f"nt{b}_{c}")
                    hh = HW // 2
                    nc.sync.dma_start(nt[:, :hh], feat_d[b, c * P:(c + 1) * P, :hh])
                    nc.sync.dma_start(nt[:, hh:], feat_d[b, c * P:(c + 1) * P, hh:])
                    for kq in range(KC // 4):
                        pft = pst.tile(
                            [P, 4, P], F32, tag="pft", bufs=5, name=f"pft{b}_{c}_{kq}"
                        )
                        for j in range(4):
                            k = kq * 4 + j
                            nc.tensor.transpose(
                                tr_cast(pft[:, j, :]),
                                tr_cast(nt[:, k * P:(k + 1) * P]),
                                tr_cast(ident),
                            )
                        # split each quad copy across DVE and ACT: halves the
                        # critical-path latency per quad
                        nc.vector.tensor_copy(
                            featT[:, kq * 4:kq * 4 + 2, c * P:(c + 1) * P],
                            pft[:, 0:2, :],
                        )
                        nc.scalar.copy(
                            featT[:, kq * 4 + 2:kq * 4 + 4, c * P:(c + 1) * P],
                            pft[:, 2:4, :],
                        )

            def phase_att(b):
                # sigmoid, DMA attentions out, PE-transpose att to [HW, M]
                raw = attp.tile([M, HW], F32, tag="raw", bufs=2, name=f"raw{b}")
                nc.scalar.dma_start(raw, att_d[b])
                asl = att_sb[:, b * HW:(b + 1) * HW]
                nc.scalar.activation(asl, raw, Sig)
                nc.scalar.dma_start(at_d[b], asl)
                patt = pst.tile([P, KC, M], F32, tag="patt", bufs=1, name=f"patt{b}")
                for k in range(KC):
                    nc.tensor.transpose(
                        tr_cast(patt[:, k, :]),
                        tr_cast(asl[:, k * P:(k + 1) * P]),
                        tr_cast(ident[:M, :M]),
                    )
                nc.vector.tensor_copy(attT[:, b], patt)

            def phase_matmul(b):
                # last sample's output DMA goes via the SP queue, which is idle
                # by then (input loads done) -> minimal enqueue latency
                out_eng = nc.sync if b == BPC - 1 else nc.scalar
                featT = featTs[b]
                out_sb = outp.tile([M, C], F32, tag="out_sb", name=f"out_sb{b}")
                for n in range(NT2):
                    ps = psm.tile([M, NMM], F32, tag="mm", bufs=2, name=f"mm{b}_{n}")
                    for k in range(KC):
                        nc.tensor.matmul(
                            ps,
                            attT[:, b, k, :],
                            featT[:, k, n * NMM:(n + 1) * NMM],
                            start=(k == 0),
                            stop=(k == KC - 1),
                        )
                    nc.scalar.activation(
                        out_sb[:, n * NMM:(n + 1) * NMM], ps, Copy, scale=1.0 / HW
                    )
                    if n % 2 == 1:
                        out_eng.dma_start(
                            fm_d[b, :, (n - 1) * NMM:(n + 1) * NMM],
                            out_sb[:, (n - 1) * NMM:(n + 1) * NMM],
                        )

            def body(_i=None):
                phase_load_transpose(0)
                phase_att(0)
                phase_att(1)
                phase_matmul(0)
                phase_load_transpose(1)
                phase_matmul(1)

            if repeat == 1:
                body()
            else:
                # timing mode: run the whole kernel `repeat` times on-device so
                # a single execution amortizes the RPC overhead
                with tc.For_i(0, repeat, 1) as _i:
                    body(_i)

    nc.compile()
    return nc


_NC_CACHE: dict = {}


def _get_nc(mm_mode: str | None = None, tr_mode: str | None = None) -> bass.Bass:
    # read module globals at call time so tests can override MM_MODE/TR_MODE
    key = (mm_mode or MM_MODE, tr_mode or TR_MODE)
    if key not in _NC_CACHE:
        _NC_CACHE[key] = _build(*key)
    return _NC_CACHE[key]


def kernel(features: np.ndarray, raw_attentions: np.ndarray):
    """Full-input entry point: shards over 8 cores, returns full outputs."""
    features = np.ascontiguousarray(features, dtype=np.float32)
    raw_attentions = np.ascontiguousarray(raw_attentions, dtype=np.float32)
    assert features.shape == (B, C, H, W)
    assert raw_attentions.shape == (B, M, H, W)

    nc = _get_nc()
    in_maps = [
        {
            "features": features[i * BPC:(i + 1) * BPC].reshape(BPC, C, HW),
            "raw_attentions": raw_attentions[i * BPC:(i + 1) * BPC].reshape(
                BPC, M, HW
            ),
        }
        for i in range(NCORES)
    ]
    res = run_bass_kernel_spmd(nc, in_maps, core_ids=list(range(NCORES)))
    fm = np.concatenate(
        [res.results[i]["feature_matrix"] for i in range(NCORES)], axis=0
    )
    att = np.concatenate(
        [res.results[i]["attentions"] for i in range(NCORES)], axis=0
    )
    return fm.reshape(B, M, C), att.reshape(B, M, H, W)
